# revision 2
# baseline (speedup 1.0000x reference)
"""Embedding-lookup kernel for 8 TRN2 NeuronCores.

Computes out[b, :] = z[b, :] + a[:, idx[b]] * scale[b] for B=1M rows.

Strategy (data-parallel over batch):
  - Each of the 8 cores handles BC = B/8 = 131072 rows; the small
    (512, 128) table a.T is replicated to every core's DRAM.
  - Per 8192-row chunk: dma_gather (GPSIMD custom DMA instruction)
    pulls the 512-byte table rows keyed by int16 indices straight from
    DRAM into SBUF, batch-wrapped across the 128 partitions; z streams
    in contiguously; DVE fuses (g * scale + z) via scalar_tensor_tensor;
    the result streams back out.
  - Indices are pre-permuted on the host so the gather's fixed
    (i%128, i//128) output layout corresponds to batch row p*T + t,
    which makes the z / out DMAs contiguous 32KB-per-partition runs.

Raw Bass (no Tile framework — the walrus build here can't encode Tile's
EVENT_SEMAPHORE_RANGE_CLEAR kernel tail), manually triple-buffered:
SP issues HWDGE loads/stores, GPSIMD the gathers, DVE the fused FMA.
Semaphores count monotonically; NRT's sema_reset preamble re-zeroes
them before every execution.

Measured (8 cores, interleaved repeat-count slope on HW): ~1.1-1.2 ms
per pass, bit-exact vs the fp32 reference (relative error 0.0). An
ablation replacing the gather with a same-volume contiguous copy is
substantially faster, so the random 512-byte table reads are the
bottleneck; splitting into 1024-index single-packet gathers made it
worse (+240 us of per-instruction SWDGE overhead), and table
replication across 16 DRAM copies (TREP) measured neutral on this
machine (kept: it cannot hurt, and should help where two NeuronCores
share an HBM stack). A dma_gather with num_idxs > 1024 needs
single_packet=False (64-descriptor-per-engine packet limit).
"""

import numpy as np

import concourse.bass as bass
import concourse.mybir as mybir
from concourse import library_config
from concourse.bass_utils import run_bass_kernel_spmd

F32 = mybir.dt.float32
I16 = mybir.dt.int16

B = 1048576
Z = 128
K = 512
NCORES = 8
BC = B // NCORES  # rows per core
NBUF = 3
# Replicate the 256KB table TREP times in DRAM and salt indices with
# i%TREP * K so the gather's random 512B reads spread across more HBM
# banks. Measured neutral on hardware where each NC has its own stack;
# kept because it cannot hurt and should help when two NCs share one.
# Salted idx must stay within int16: TREP*K <= 32767.
TREP = 16


def build_program(bc=BC, chunk=8192, repeats=1, gather_n=8192, _ablate=(), nbuf=None, nqueues=1, scratch=16384):
    """Build the single-core Bass program (same module runs SPMD on all cores).

    repeats > 1 re-runs the whole computation (statically unrolled) for
    benchmarking: wall-time slope over repeats isolates on-device time.
    gather_n: indices per dma_gather instruction. <=1024 fits the 64-
    descriptor-per-engine single-packet limit; larger needs multi-packet.
    """
    t = chunk // 128  # column blocks per chunk
    nch = bc // chunk  # chunks per core
    assert bc % chunk == 0 and chunk % 128 == 0
    NBUF = nbuf or globals()["NBUF"]
    gather_n = min(gather_n, chunk)
    nsub = chunk // gather_n
    assert nsub % nqueues == 0 or nqueues == 1
    spq = max(nsub // nqueues, 1)  # gathers per queue per chunk
    assert chunk % gather_n == 0 and gather_n % 128 == 0
    total = nch * repeats

    nc = bass.Bass(num_swdge_queues=nqueues, dynamic_dma_scratch_size=scratch)
    z = nc.declare_dram_parameter("z", [bc, Z], F32, isOutput=False)
    at = nc.declare_dram_parameter("at", [K * TREP, Z], F32, isOutput=False)
    idxw = nc.declare_dram_parameter("idxw", [nch, 128, chunk // 16], I16, isOutput=False)
    scw = nc.declare_dram_parameter("scw", [nch, 128, t], F32, isOutput=False)
    out = nc.declare_dram_parameter("out", [bc, Z], F32, isOutput=True)

    # chunk-row b = p*t + tt lives at SBUF (partition p, column block tt)
    z_v = z.ap().rearrange("(c p tt) d -> c p (tt d)", p=128, tt=t)
    o_v = out.ap().rearrange("(c p tt) d -> c p (tt d)", p=128, tt=t)

    import contextlib

    with contextlib.ExitStack() as ctx:
        zts = [
            ctx.enter_context(nc.sbuf_tensor(f"zt{i}", [128, t * Z], F32))
            for i in range(NBUF)
        ]
        gts = [
            ctx.enter_context(nc.sbuf_tensor(f"gt{i}", [128, t, Z], F32))
            for i in range(NBUF)
        ]
        idxts = [
            ctx.enter_context(nc.sbuf_tensor(f"idxt{i}", [128, chunk // 16], I16))
            for i in range(NBUF)
        ]
        scts = [
            ctx.enter_context(nc.sbuf_tensor(f"sct{i}", [128, t], F32))
            for i in range(NBUF)
        ]
        # DMA semaphores are per buffer slot: a slot's next use is gated on
        # the previous use's consumers, so at most one chunk's DMAs are ever
        # in flight per semaphore and thresholds cannot be satisfied by a
        # later DMA's partial increments.
        sem_is = [ctx.enter_context(nc.semaphore(f"sem_is{i}")) for i in range(NBUF)]
        sem_z = [ctx.enter_context(nc.semaphore(f"sem_z{i}")) for i in range(NBUF)]
        sem_g = [
            [
                ctx.enter_context(nc.semaphore(f"sem_g{i}q{q}"))
                for q in range(nqueues)
            ]
            for i in range(NBUF)
        ]
        sem_o = [ctx.enter_context(nc.semaphore(f"sem_o{i}")) for i in range(NBUF)]
        sem_v = ctx.enter_context(nc.semaphore("sem_v"))  # DVE chunk done, +1/chunk
        block = ctx.enter_context(nc.Block())

        def nuses(j):  # completed uses of slot j%NBUF's sems after chunk j
            return j // NBUF + 1

        @block.sync
        def _(sync):
            for k in range(total):
                c = k % nch
                b = k % NBUF
                if k >= NBUF:
                    # slot reuse: gather(k-NBUF), DVE(k-NBUF), out(k-NBUF) done
                    for q in range(nqueues):
                        sync.wait_ge(sem_g[b][q], 16 * spq * nuses(k - NBUF))
                    sync.wait_ge(sem_v, k - NBUF + 1)
                    sync.wait_ge(sem_o[b], 16 * nuses(k - NBUF))
                sync.dma_start(out=idxts[b][:], in_=idxw.ap()[c]).then_inc(sem_is[b], 16)
                sync.dma_start(out=scts[b][:], in_=scw.ap()[c]).then_inc(sem_is[b], 16)
                sync.dma_start(out=zts[b][:], in_=z_v[c]).then_inc(sem_z[b], 16)
                if k >= 2:
                    j = k - 2  # store lags loads by 2 chunks
                    sync.wait_ge(sem_v, j + 1)
                    sync.dma_start(out=o_v[j % nch], in_=zts[j % NBUF][:]).then_inc(
                        sem_o[j % NBUF], 16
                    )
            for j in range(max(total - 2, 0), total):
                sync.wait_ge(sem_v, j + 1)
                sync.dma_start(out=o_v[j % nch], in_=zts[j % NBUF][:]).then_inc(
                    sem_o[j % NBUF], 16
                )
            for b in range(NBUF):
                count_b = len([j for j in range(total) if j % NBUF == b])
                if count_b:
                    sync.wait_ge(sem_o[b], 16 * count_b)

        @block.gpsimd
        def _(gpsimd):
            gpsimd.load_library(library_config.mlp)
            creg = gpsimd.to_reg(gather_n)
            ts = gather_n // 128  # column blocks per sub-gather
            for k in range(total):
                b = k % NBUF
                c = k % nch
                gpsimd.wait_ge(sem_is[b], 32 * nuses(k))
                if k >= NBUF:
                    gpsimd.wait_ge(sem_v, k - NBUF + 1)  # gt slot reuse
                for s in range(nsub):
                    if "nogather" in _ablate:
                        # same bytes, contiguous SWDGE read instead of gather
                        gpsimd.dma_start(
                            out=gts[b][:, s * ts : (s + 1) * ts, :],
                            in_=z_v[c][:, s * ts * Z : (s + 1) * ts * Z],
                        ).then_inc(sem_g[b][0], 16)
                        continue
                    gpsimd.dma_gather(
                        out_ap=gts[b][:, s * ts : (s + 1) * ts, :],
                        in_ap=at.ap(),
                        idxs_ap=idxts[b][:, s * (gather_n // 16) : (s + 1) * (gather_n // 16)],
                        num_idxs=gather_n,
                        num_idxs_reg=creg,
                        elem_size=Z,
                        # >64 descriptors per SDMA engine (num_idxs > 1024)
                        # exceeds the single-packet limit on HW
                        single_packet=(gather_n <= 1024),
                        queue_num=s % nqueues,
                    ).then_inc(sem_g[b][s % nqueues], 16)

        @block.vector
        def _(vector):
            for k in range(total):
                b = k % NBUF
                for q in range(nqueues):
                    vector.wait_ge(sem_g[b][q], 16 * spq * nuses(k))
                vector.wait_ge(sem_z[b], 16 * nuses(k))
                vector.wait_ge(sem_is[b], 32 * nuses(k))
                if k >= NBUF:
                    vector.wait_ge(sem_o[b], 16 * nuses(k - NBUF))  # zt rewrite vs out read
                nt = 1 if "nodve" in _ablate else t
                for tt in range(nt):
                    inst = vector.scalar_tensor_tensor(
                        out=zts[b][:, tt * Z : (tt + 1) * Z],
                        in0=gts[b][:, tt, :],
                        scalar=scts[b][:, tt : tt + 1],
                        in1=zts[b][:, tt * Z : (tt + 1) * Z],
                        op0=mybir.AluOpType.mult,
                        op1=mybir.AluOpType.add,
                    )
                inst.then_inc(sem_v, 1)

    # Raw Bass skips Bacc's extended-inst lowering; without it the NEFF
    # compiler sees empty .instr on InstISA subclasses -> "ISA wrong length".
    mybir.codegen_inst_isa_subclasses(nc)
    return nc


def prep_core_inputs(z, at, idx16, scale, bc, chunk):
    """Host-side layout prep for one core's batch slice."""
    t = chunk // 128
    nch = bc // chunk
    # gather position i -> batch row (i%128)*t + i//128 within the chunk
    i = np.arange(chunk)
    perm = (i % 128) * t + i // 128
    pres = idx16.reshape(nch, chunk)[:, perm]  # [nch, chunk]
    if TREP > 1:
        # spread reads across TREP table copies by presented position
        pres = pres + ((i % TREP) * K).astype(np.int16)[None, :]
    # gather reads index i from (partition i%16, column i//16); replicate x8
    idxw = np.tile(pres.reshape(nch, chunk // 16, 16).transpose(0, 2, 1), (1, 8, 1))
    scw = scale.reshape(nch, 128, t)
    return {
        "z": np.ascontiguousarray(z),
        "at": at,
        "idxw": np.ascontiguousarray(idxw),
        "scw": np.ascontiguousarray(scw),
    }


def prep_all_cores(z, a, labels_idx, labels_scale, _chunk=8192):
    at = np.ascontiguousarray(np.tile(np.asarray(a).T, (TREP, 1)))
    idx16 = np.asarray(labels_idx).astype(np.int16)
    z = np.asarray(z)
    labels_scale = np.asarray(labels_scale)
    ins = []
    for m in range(NCORES):
        s = slice(m * BC, (m + 1) * BC)
        ins.append(prep_core_inputs(z[s], at, idx16[s], labels_scale[s], BC, _chunk))
    return ins


def kernel(z, a, labels_idx, labels_scale, _chunk=8192, _trace=False):
    nc = build_program(BC, _chunk)
    ins = prep_all_cores(z, a, labels_idx, labels_scale, _chunk)
    res = run_bass_kernel_spmd(nc, ins, core_ids=list(range(NCORES)), trace=_trace)
    full = np.concatenate([res.results[m]["out"] for m in range(NCORES)], axis=0)
    if _trace:
        return full, res
    return full



# revision 5
# speedup vs baseline: 348.4521x; 348.4521x over previous
"""Embedding-lookup kernel for 8 TRN2 NeuronCores.

Computes out[b, :] = z[b, :] + a[:, idx[b]] * scale[b] for B=1M rows.

Strategy (data-parallel over batch):
  - Each of the 8 cores handles BC = B/8 = 131072 rows; the small
    (512, 128) table a.T is replicated to every core's DRAM.
  - Per 8192-row chunk: dma_gather (GPSIMD custom DMA instruction)
    pulls the 512-byte table rows keyed by int16 indices straight from
    DRAM into SBUF, batch-wrapped across the 128 partitions; z streams
    in contiguously; DVE fuses (g * scale + z) via scalar_tensor_tensor;
    the result streams back out.
  - Indices are pre-permuted on the host so the gather's fixed
    (i%128, i//128) output layout corresponds to batch row p*T + t,
    which makes the z / out DMAs contiguous 32KB-per-partition runs.

Raw Bass (no Tile framework — the walrus build here can't encode Tile's
EVENT_SEMAPHORE_RANGE_CLEAR kernel tail), manually triple-buffered:
SP issues HWDGE loads/stores, GPSIMD the gathers, DVE the fused FMA.
Semaphores count monotonically; NRT's sema_reset preamble re-zeroes
them before every execution.

Measured (8 cores, interleaved repeat-count slope on HW): ~1.1-1.2 ms
per pass, bit-exact vs the fp32 reference (relative error 0.0). An
ablation replacing the gather with a same-volume contiguous copy is
substantially faster, so the random 512-byte table reads are the
bottleneck; splitting into 1024-index single-packet gathers made it
worse (+240 us of per-instruction SWDGE overhead), and table
replication across 16 DRAM copies (TREP) measured neutral on this
machine (kept: it cannot hurt, and should help where two NeuronCores
share an HBM stack). A dma_gather with num_idxs > 1024 needs
single_packet=False (64-descriptor-per-engine packet limit).
"""

import numpy as np

import concourse.bass as bass
import concourse.mybir as mybir
from concourse import library_config
from concourse.bass_utils import run_bass_kernel_spmd

F32 = mybir.dt.float32
I16 = mybir.dt.int16

B = 1048576
Z = 128
K = 512
NCORES = 8
BC = B // NCORES  # rows per core
NBUF = 3
# Replicate the 256KB table TREP times in DRAM and salt indices with
# i%TREP * K so the gather's random 512B reads spread across more HBM
# banks. Measured neutral on hardware where each NC has its own stack;
# kept because it cannot hurt and should help when two NCs share one.
# Salted idx must stay within int16: TREP*K <= 32767.
TREP = 16


def build_program(bc=BC, chunk=8192, repeats=1, gather_n=8192, _ablate=(), nbuf=None, nqueues=1, scratch=16384, bench_io=False):
    """Build the single-core Bass program (same module runs SPMD on all cores).

    repeats > 1 re-runs the whole computation (statically unrolled) for
    benchmarking: wall-time slope over repeats isolates on-device time.
    gather_n: indices per dma_gather instruction. <=1024 fits the 64-
    descriptor-per-engine single-packet limit; larger needs multi-packet.
    bench_io: timing-only variant — z/at/out become Internal DRAM scratch
    (same instruction stream, garbage data) so per-execution host<->device
    transfers shrink from ~1.5GB to ~20MB; a tiny `done` external output
    provides completion. Numerically meaningless, structurally identical.
    """
    t = chunk // 128  # column blocks per chunk
    nch = bc // chunk  # chunks per core
    assert bc % chunk == 0 and chunk % 128 == 0
    NBUF = nbuf or globals()["NBUF"]
    gather_n = min(gather_n, chunk)
    nsub = chunk // gather_n
    assert nsub % nqueues == 0 or nqueues == 1
    spq = max(nsub // nqueues, 1)  # gathers per queue per chunk
    assert chunk % gather_n == 0 and gather_n % 128 == 0
    total = nch * repeats

    nc = bass.Bass(num_swdge_queues=nqueues, dynamic_dma_scratch_size=scratch)
    if bench_io:
        z = nc.dram_tensor("z", [bc, Z], F32, kind="Internal")
        at = nc.dram_tensor("at", [K * TREP, Z], F32, kind="Internal")
        idxw = nc.declare_dram_parameter("idxw", [nch, 128, chunk // 16], I16, isOutput=False)
        scw = nc.declare_dram_parameter("scw", [nch, 128, t], F32, isOutput=False)
        out = nc.dram_tensor("out", [bc, Z], F32, kind="Internal")
        done = nc.declare_dram_parameter("done", [1, 64], I16, isOutput=True)
    else:
        z = nc.declare_dram_parameter("z", [bc, Z], F32, isOutput=False)
        at = nc.declare_dram_parameter("at", [K * TREP, Z], F32, isOutput=False)
        idxw = nc.declare_dram_parameter("idxw", [nch, 128, chunk // 16], I16, isOutput=False)
        scw = nc.declare_dram_parameter("scw", [nch, 128, t], F32, isOutput=False)
        out = nc.declare_dram_parameter("out", [bc, Z], F32, isOutput=True)
        done = None

    # chunk-row b = p*t + tt lives at SBUF (partition p, column block tt)
    z_v = z.ap().rearrange("(c p tt) d -> c p (tt d)", p=128, tt=t)
    o_v = out.ap().rearrange("(c p tt) d -> c p (tt d)", p=128, tt=t)

    import contextlib

    with contextlib.ExitStack() as ctx:
        zts = [
            ctx.enter_context(nc.sbuf_tensor(f"zt{i}", [128, t * Z], F32))
            for i in range(NBUF)
        ]
        gts = [
            ctx.enter_context(nc.sbuf_tensor(f"gt{i}", [128, t, Z], F32))
            for i in range(NBUF)
        ]
        idxts = [
            ctx.enter_context(nc.sbuf_tensor(f"idxt{i}", [128, chunk // 16], I16))
            for i in range(NBUF)
        ]
        scts = [
            ctx.enter_context(nc.sbuf_tensor(f"sct{i}", [128, t], F32))
            for i in range(NBUF)
        ]
        # DMA semaphores are per buffer slot: a slot's next use is gated on
        # the previous use's consumers, so at most one chunk's DMAs are ever
        # in flight per semaphore and thresholds cannot be satisfied by a
        # later DMA's partial increments.
        sem_is = [ctx.enter_context(nc.semaphore(f"sem_is{i}")) for i in range(NBUF)]
        sem_z = [ctx.enter_context(nc.semaphore(f"sem_z{i}")) for i in range(NBUF)]
        sem_g = [
            [
                ctx.enter_context(nc.semaphore(f"sem_g{i}q{q}"))
                for q in range(nqueues)
            ]
            for i in range(NBUF)
        ]
        sem_o = [ctx.enter_context(nc.semaphore(f"sem_o{i}")) for i in range(NBUF)]
        sem_v = ctx.enter_context(nc.semaphore("sem_v"))  # DVE chunk done, +1/chunk
        block = ctx.enter_context(nc.Block())

        def nuses(j):  # completed uses of slot j%NBUF's sems after chunk j
            return j // NBUF + 1

        @block.sync
        def _(sync):
            for k in range(total):
                c = k % nch
                b = k % NBUF
                if k >= NBUF:
                    # slot reuse: gather(k-NBUF), DVE(k-NBUF), out(k-NBUF) done
                    for q in range(nqueues):
                        sync.wait_ge(sem_g[b][q], 16 * spq * nuses(k - NBUF))
                    sync.wait_ge(sem_v, k - NBUF + 1)
                    sync.wait_ge(sem_o[b], 16 * nuses(k - NBUF))
                sync.dma_start(out=idxts[b][:], in_=idxw.ap()[c]).then_inc(sem_is[b], 16)
                sync.dma_start(out=scts[b][:], in_=scw.ap()[c]).then_inc(sem_is[b], 16)
                sync.dma_start(out=zts[b][:], in_=z_v[c]).then_inc(sem_z[b], 16)
                if k >= 2:
                    j = k - 2  # store lags loads by 2 chunks
                    sync.wait_ge(sem_v, j + 1)
                    sync.dma_start(out=o_v[j % nch], in_=zts[j % NBUF][:]).then_inc(
                        sem_o[j % NBUF], 16
                    )
            for j in range(max(total - 2, 0), total):
                sync.wait_ge(sem_v, j + 1)
                sync.dma_start(out=o_v[j % nch], in_=zts[j % NBUF][:]).then_inc(
                    sem_o[j % NBUF], 16
                )
            for b in range(NBUF):
                count_b = len([j for j in range(total) if j % NBUF == b])
                if count_b:
                    sync.wait_ge(sem_o[b], 16 * count_b)
            if done is not None:
                sync.dma_start(out=done.ap(), in_=idxts[0][:1, :64]).then_inc(
                    sem_is[0], 16
                )
                sync.wait_ge(sem_is[0], 32 * nuses(total - 1) + 16)

        @block.gpsimd
        def _(gpsimd):
            gpsimd.load_library(library_config.mlp)
            creg = gpsimd.to_reg(gather_n)
            ts = gather_n // 128  # column blocks per sub-gather
            for k in range(total):
                b = k % NBUF
                c = k % nch
                gpsimd.wait_ge(sem_is[b], 32 * nuses(k))
                if k >= NBUF:
                    gpsimd.wait_ge(sem_v, k - NBUF + 1)  # gt slot reuse
                for s in range(nsub):
                    if "nogather" in _ablate:
                        # same bytes, contiguous SWDGE read instead of gather
                        gpsimd.dma_start(
                            out=gts[b][:, s * ts : (s + 1) * ts, :],
                            in_=z_v[c][:, s * ts * Z : (s + 1) * ts * Z],
                        ).then_inc(sem_g[b][0], 16)
                        continue
                    gpsimd.dma_gather(
                        out_ap=gts[b][:, s * ts : (s + 1) * ts, :],
                        in_ap=at.ap(),
                        idxs_ap=idxts[b][:, s * (gather_n // 16) : (s + 1) * (gather_n // 16)],
                        num_idxs=gather_n,
                        num_idxs_reg=creg,
                        elem_size=Z,
                        # >64 descriptors per SDMA engine (num_idxs > 1024)
                        # exceeds the single-packet limit on HW
                        single_packet=(gather_n <= 1024),
                        queue_num=s % nqueues,
                    ).then_inc(sem_g[b][s % nqueues], 16)

        @block.vector
        def _(vector):
            for k in range(total):
                b = k % NBUF
                for q in range(nqueues):
                    vector.wait_ge(sem_g[b][q], 16 * spq * nuses(k))
                vector.wait_ge(sem_z[b], 16 * nuses(k))
                vector.wait_ge(sem_is[b], 32 * nuses(k))
                if k >= NBUF:
                    vector.wait_ge(sem_o[b], 16 * nuses(k - NBUF))  # zt rewrite vs out read
                nt = 1 if "nodve" in _ablate else t
                for tt in range(nt):
                    inst = vector.scalar_tensor_tensor(
                        out=zts[b][:, tt * Z : (tt + 1) * Z],
                        in0=gts[b][:, tt, :],
                        scalar=scts[b][:, tt : tt + 1],
                        in1=zts[b][:, tt * Z : (tt + 1) * Z],
                        op0=mybir.AluOpType.mult,
                        op1=mybir.AluOpType.add,
                    )
                inst.then_inc(sem_v, 1)

    # Raw Bass skips Bacc's extended-inst lowering; without it the NEFF
    # compiler sees empty .instr on InstISA subclasses -> "ISA wrong length".
    mybir.codegen_inst_isa_subclasses(nc)
    return nc


def prep_core_inputs(z, at, idx16, scale, bc, chunk):
    """Host-side layout prep for one core's batch slice."""
    t = chunk // 128
    nch = bc // chunk
    # gather position i -> batch row (i%128)*t + i//128 within the chunk
    i = np.arange(chunk)
    perm = (i % 128) * t + i // 128
    pres = idx16.reshape(nch, chunk)[:, perm]  # [nch, chunk]
    if TREP > 1:
        # spread reads across TREP table copies by presented position
        pres = pres + ((i % TREP) * K).astype(np.int16)[None, :]
    # gather reads index i from (partition i%16, column i//16); replicate x8
    idxw = np.tile(pres.reshape(nch, chunk // 16, 16).transpose(0, 2, 1), (1, 8, 1))
    scw = scale.reshape(nch, 128, t)
    return {
        "z": np.ascontiguousarray(z),
        "at": at,
        "idxw": np.ascontiguousarray(idxw),
        "scw": np.ascontiguousarray(scw),
    }


def prep_all_cores(z, a, labels_idx, labels_scale, _chunk=8192):
    at = np.ascontiguousarray(np.tile(np.asarray(a).T, (TREP, 1)))
    idx16 = np.asarray(labels_idx).astype(np.int16)
    z = np.asarray(z)
    labels_scale = np.asarray(labels_scale)
    ins = []
    for m in range(NCORES):
        s = slice(m * BC, (m + 1) * BC)
        ins.append(prep_core_inputs(z[s], at, idx16[s], labels_scale[s], BC, _chunk))
    return ins


def kernel(z, a, labels_idx, labels_scale, _chunk=8192, _trace=False):
    nc = build_program(BC, _chunk)
    ins = prep_all_cores(z, a, labels_idx, labels_scale, _chunk)
    res = run_bass_kernel_spmd(nc, ins, core_ids=list(range(NCORES)), trace=_trace)
    full = np.concatenate([res.results[m]["out"] for m in range(NCORES)], axis=0)
    if _trace:
        return full, res
    return full

